# revision 60
# baseline (speedup 1.0000x reference)
"""Multi-head attention on 8 TRN2 NeuronCores (Bass/Tile).

Problem: B=4, S=2048, D=1024, H=16 heads (DH=64).
  out = softmax((q@wq+bq)(k@wk+bk)^T / sqrt(H)) @ (v@wv+bv) @ wo + bo

Sharding: 8 cores = 4 batches x 2 head-groups (8 heads each). Each core
computes its batch's QKV projections restricted to its head group's
columns, attention for those 8 heads, and a partial output projection
(wo rows for its heads); the host sums 3 per-ktile-group partials per
batch. Activations are kept TRANSPOSED ([feature, seq]) on device so
every matmul has its contraction on the partition dim; the host
transposes inputs/outputs (cheap numpy).

v2: PE is the critical engine (~330us busy of matmul work that cannot
shrink at >=16-bit dtypes: QK 109us + PV 109us + QKV projections 82us
+ WO 27us), so the schedule exists to keep PE saturated end-to-end:
  prefix (~40us, paced by bf16 input DMA): vh (all 16 ktiles, all 8
    heads, + ones col) -> khT m-tile 0 chunk 0 -> qhT m-tile 0 chunk 0.
  attention stream, blocks = head x q-window in order
  [0,1,2,3,4,5,7,6] x 2 windows so the LAST block's normalize is the
  direct-write hb=0 path (no gpsimd staging hop on the critical tail):
    scT = khT_h^T @ qhT_h (K=64, PSUM [128,1024]); eT = exp(scT/4)
    (bf16, one ACT instr, ~1.04us = the ACT pace); ctx_ext[65,512] +=
    vh^T @ eT (ones col -> row 64 = softmax sums); normalize via DVE
    reciprocal + gpsimd partition_broadcast + DVE mul -> ctxT bf16.
  Each step pops background PE work into the stream (pop_bg): first
  the remaining projection chains (khT/qhT m0-chunk1 then m-tiles 1-3,
  2 pops/step), then wo partial chains (1/step, bank-cycle limited)
  sub-gated per window-half on normalize completion. Emission-order
  deadlines are hard: PE executes in order, so every chain matmul must
  be EMITTED before the attention matmul that reads its output; with
  2 pops/step all proj chains are emitted by step ~104 vs first-reader
  deadlines {8,16,64,128,192}. Matmul outputs never cross a PSUM bank
  (max 512 f32 cols per accumulation chain).
  Inputs kT/qT stay resident in SBUF (64KB/part) so chains need no
  reloads. All activations and weights are bf16 (PSUM accumulation
  f32); partial outputs outA=wo[kt0]+wo[kt1] (accumulated in PSUM
  before one bf16 round), out2, out3; bv/bo fold exactly through the
  linear tail on the host since softmax rows sum to 1.
  Tail: the 16 post-attention kt3 chains rotate across all 6 freed
  PSUM slots with drains alternating ACT/DVE across two SBUF rings.

The PV matmuls run with a 2-step software-pipeline skew behind the
exps (hardware-validated; depth 4 races on silicon despite simulating
fine, and depth 2 sims 1.7us faster than 3). The ctx ring stays at
3 banks (ring 2 sims 30us slower from PV stalls on normalize).

Measured (8-core SPMD, axon): rel err vs fp32 reference 8.56e-3
(budget 2e-2; bf16 inputs/weights dominate); cost-model timeline
369.4us/core vs 421.6us for the v1 phase-separated schedule
(PE busy ~330/370 = 89%; remaining idle: ~10us DMA-paced prefix,
~10us scattered pop/dead-window bubbles, ~8us tail fin+drain flush).
"""
import ml_dtypes
import numpy as np

import concourse.mybir as mybir
from concourse import bacc
from concourse.tile import TileContext
from concourse.bass_utils import run_bass_kernel_spmd

B, S, D, H = 4, 2048, 1024, 16
DH = D // H          # 64
HG = H // 2          # 8 heads per core
DL = HG * DH         # 512 local qkv width
KT = D // 128        # 8 contraction tiles for projections
ST = S // 128        # 16 key tiles
QW = S // 1024       # 2 q windows of 1024
SC = S // 1024       # 2 s-chunks for input tiles
SCALE = 1.0 / np.sqrt(np.float32(H))  # 0.25

f32 = mybir.dt.float32
bf16 = mybir.dt.bfloat16


def _build_program() -> bacc.Bacc:
    nc = bacc.Bacc()
    qT_e = nc.declare_dram_parameter("qT", [D, S], bf16, isOutput=False)
    kT_e = nc.declare_dram_parameter("kT", [D, S], bf16, isOutput=False)
    vT_e = nc.declare_dram_parameter("vT", [D, S], bf16, isOutput=False)
    wq_e = nc.declare_dram_parameter("wq", [D, DL], bf16, isOutput=False)
    wk_e = nc.declare_dram_parameter("wk", [D, DL], bf16, isOutput=False)
    wv_e = nc.declare_dram_parameter("wv", [D, DL], bf16, isOutput=False)
    wo_e = nc.declare_dram_parameter("wo", [DL, D], bf16, isOutput=False)
    bq_e = nc.declare_dram_parameter("bq", [DL], f32, isOutput=False)
    bk_e = nc.declare_dram_parameter("bk", [DL], f32, isOutput=False)
    # partial outputs: kt0+kt1 merged (chains accumulate both in PSUM
    # before one drain), kt2 and kt3 separate for earlier gating
    outA_e = nc.declare_dram_parameter("outA", [D, S], bf16, isOutput=True)
    out2_e = nc.declare_dram_parameter("out2", [D, S], bf16, isOutput=True)
    out3_e = nc.declare_dram_parameter("out3", [D, S], bf16, isOutput=True)

    with TileContext(nc) as tc:
        with (
            tc.tile_pool(name="wp", bufs=1) as wpool,
            tc.tile_pool(name="kin", bufs=1) as kinpool,
            tc.tile_pool(name="qin", bufs=1) as qinpool,
            tc.tile_pool(name="vin", bufs=1) as vinpool,
            tc.tile_pool(name="proj", bufs=1) as projpool,
            tc.tile_pool(name="attn", bufs=1) as attnpool,
            tc.tile_pool(name="sm", bufs=3) as smpool,
            tc.tile_pool(name="ps", bufs=2, space="PSUM") as pspool,
            tc.tile_pool(name="ctxps", bufs=3, space="PSUM") as ctxpspool,
            tc.tile_pool(name="bgps", bufs=1, space="PSUM") as bgpspool,
        ):
            # ---- DMA queue (SP, FIFO) is ordered by first use: biases,
            # wv, vT, wk, kT-c0, wq, qT-c0, then the bg chunk-1 inputs
            # and wo (not needed until hl4).
            bias_t = {}

            def load_biases():
                # issued after the vT stream: first use is the khT-m0
                # bias-add at ~30us, so keep them off the DMA queue head
                for nm, ext in [("bq", bq_e), ("bk", bk_e)]:
                    bias_t[nm] = wpool.tile([128, DL // 128], f32, tag=nm,
                                            name=f"b_{nm}")
                    nc.sync.dma_start(
                        out=bias_t[nm], in_=ext.rearrange("(j p) -> p j", p=128)
                    )

            def load_input_chunk(pool, nm, ext, c, xt, bufs=SC * KT):
                for t in range(KT):
                    x = pool.tile([128, 1024], bf16, tag="in", bufs=bufs,
                                  name=f"{nm}c{c}t{t}")
                    nc.sync.dma_start(
                        out=x,
                        in_=ext[t * 128:(t + 1) * 128, c * 1024:(c + 1) * 1024],
                    )
                    xt[c][t] = x

            def load_weight_input_interleaved(wnm, wext, pool, xnm, xext, c,
                                              xt, bufs=SC * KT, xq=None):
                # one wide rearranged DMA for the whole weight (1 HWDGE
                # issue instead of 8 in the critical prefix window; same
                # bytes/descriptors), then the input tiles; consumers use
                # identical [128, DL] slices of the wide tile
                xq = xq or nc.sync
                w = wpool.tile([128, KT, DL], bf16, tag=wnm, bufs=1,
                               name=f"w{wnm}")
                nc.sync.dma_start(
                    out=w, in_=wext.rearrange("(t p) d -> p t d", p=128)
                )
                wt = [w[:, t, :] for t in range(KT)]
                for t in range(KT):
                    x = pool.tile([128, 1024], bf16, tag="in", bufs=bufs,
                                  name=f"{xnm}c{c}t{t}")
                    xq.dma_start(
                        out=x,
                        in_=xext[t * 128:(t + 1) * 128, c * 1024:(c + 1) * 1024],
                    )
                    xt[c][t] = x
                return wt

            # vh first: its 27us of PE work starts ~1us in
            vxt = [[None] * KT for _ in range(SC)]
            wt_v = load_weight_input_interleaved("wv", wv_e, vinpool, "vT",
                                                 vT_e, 0, vxt, bufs=12)
            load_input_chunk(vinpool, "vT", vT_e, 1, vxt, bufs=12)
            load_biases()

            vh_tiles = []
            for c in range(SC):
                for sti in range(8):
                    st = c * 8 + sti
                    ps = pspool.tile([128, DL], f32, tag="sc", bufs=2,
                                     name=f"vps{st}")
                    for t in range(KT):
                        nc.tensor.matmul(
                            ps[:, :],
                            vxt[c][t][:, sti * 128:(sti + 1) * 128],
                            wt_v[t][:, :],
                            start=(t == 0), stop=(t == KT - 1),
                        )
                    vt = projpool.tile([128, HG, 65], bf16, tag="vh", bufs=ST,
                                       name=f"vh{st}")
                    nc.vector.tensor_copy(
                        vt[:, :, 0:64], ps.rearrange("p (h d) -> p h d", h=HG)
                    )
                    nc.vector.memset(vt[:, :, 64:65], 1.0)
                    vh_tiles.append(vt)

            khT = [
                projpool.tile([128, S], bf16, tag="khT", bufs=4, name=f"khTm{m}")
                for m in range(4)
            ]
            qhT = [
                projpool.tile([128, S], bf16, tag="qhT", bufs=4, name=f"qhTm{m}")
                for m in range(4)
            ]

            def proj_wide(tiles, wt, xt, bname, m, c):
                # one [128,1024] chunk of a projection dh-tile (prefix path);
                # matmul outputs must stay within one PSUM bank (512 f32),
                # so accumulate the two halves as separate chains
                ps = pspool.tile([128, 1024], f32, tag="sc", bufs=2,
                                 name=f"pp{bname}{m}{c}")
                for half in range(2):
                    for t in range(KT):
                        nc.tensor.matmul(
                            ps[:, half * 512:(half + 1) * 512],
                            wt[t][:, m * 128:(m + 1) * 128],
                            xt[c][t][:, half * 512:(half + 1) * 512],
                            start=(t == 0), stop=(t == KT - 1),
                        )
                nc.vector.tensor_scalar_add(
                    tiles[m][:, c * 1024:(c + 1) * 1024], ps[:, :],
                    bias_t[bname][:, m:m + 1],
                )

            def proj_bg(tiles, wt, xt, bname, m, scs):
                # background chains: [128,512] halves in the 1-bank bg pool
                for sc in scs:
                    c, half = sc // 2, sc % 2
                    state = {}

                    def emit_mm(t, c=c, half=half, m=m, state=state):
                        def go():
                            if t == 0:
                                state["ps"] = bgpspool.tile(
                                    [128, 512], f32, tag="bg", bufs=1,
                                    name=f"bp{bname}{m}{c}{half}")
                            nc.tensor.matmul(
                                state["ps"][:, :],
                                wt[t][:, m * 128:(m + 1) * 128],
                                xt[c][t][:, half * 512:(half + 1) * 512],
                                start=(t == 0), stop=(t == KT - 1),
                            )
                            if t == KT - 1:
                                nc.vector.tensor_scalar_add(
                                    tiles[m][:, sc * 512:(sc + 1) * 512],
                                    state["ps"][:, :],
                                    bias_t[bname][:, m:m + 1],
                                )
                        return go
                    for t in range(KT):
                        yield ("mm", emit_mm(t))

            # kT + khT-m0-c0, qT-c0 + qhT-m0-c0 (attention needs these)
            kxt = [[None] * KT for _ in range(SC)]
            wt_k = load_weight_input_interleaved("wk", wk_e, kinpool, "kT",
                                                 kT_e, 0, kxt)
            proj_wide(khT, wt_k, kxt, "bk", 0, 0)

            qxt = [[None] * KT for _ in range(SC)]
            wt_q = load_weight_input_interleaved("wq", wq_e, qinpool, "qT",
                                                 qT_e, 0, qxt)
            proj_wide(qhT, wt_q, qxt, "bq", 0, 0)

            # chunk-1 inputs + wo: queued behind the prefix DMAs; their
            # consumers pop in during early attention steps
            load_input_chunk(kinpool, "kT", kT_e, 1, kxt)
            load_input_chunk(qinpool, "qT", qT_e, 1, qxt)
            wo_t = [
                wpool.tile([128, D], bf16, tag="wo", bufs=4, name=f"wo{t}")
                for t in range(4)
            ]
            for t in range(4):
                nc.sync.dma_start(out=wo_t[t], in_=wo_e[t * 128:(t + 1) * 128, :])

            # ctxT: heads stacked on partitions, 4 tiles of [128, S]
            ctxT = [
                attnpool.tile([128, S], bf16, tag="ctxT", bufs=4, name=f"ctxT{t}")
                for t in range(4)
            ]

            def wo_chain(m, sc, kts, out_ext, tag, use_act=False,
                         use_ctxps=False):
                # output-projection chain accumulating the given ctxT
                # k-tiles -> one partial drain
                if use_ctxps:
                    ps = ctxpspool.tile([128, 512], f32, tag="ctx", bufs=3,
                                        name=f"bg{tag}{m}{sc}")
                else:
                    ps = bgpspool.tile([128, 512], f32, tag="bg", bufs=1,
                                       name=f"bg{tag}{m}{sc}")
                for i, kt in enumerate(kts):
                    yield ("mm", lambda ps=ps, kt=kt, i=i: nc.tensor.matmul(
                        ps[:, :],
                        wo_t[kt][:, m * 128:(m + 1) * 128],
                        ctxT[kt][:, sc * 512:(sc + 1) * 512],
                        start=(i == 0), stop=(i == len(kts) - 1),
                    ))

                def drain(ps=ps, m=m, sc=sc):
                    ot = smpool.tile([128, 512], bf16, tag="ot", bufs=4,
                                     name=f"ot{tag}{m}{sc}")
                    if use_act:
                        nc.scalar.copy(ot[:, :], ps[:, :])
                    else:
                        nc.vector.tensor_copy(ot[:, :], ps[:, :])
                    nc.sync.dma_start(
                        out=out_ext[m * 128:(m + 1) * 128,
                                    sc * 512:(sc + 1) * 512],
                        in_=ot[:, :],
                    )
                yield ("dma", drain)

            def attention(vh_tiles, qhT, khT, bgs=()):
                pending_pv = []  # (emit_fn, finalize_or_None)

                def flush_one_pv(depth=2):
                    if len(pending_pv) >= depth:
                        emit, fin = pending_pv.pop(0)
                        emit()
                        if fin is not None:
                            fin()

                def pop_bg(bi):
                    budget = 2
                    for ent in bgs:
                        if bi < ent[0] or ent[1] is None:
                            continue
                        if ent[2] > 0:
                            ent[2] -= 1
                            return
                        while budget > 0:
                            try:
                                kind, go = next(ent[1])
                            except StopIteration:
                                ent[1] = None
                                break
                            go()
                            if kind == "mm":
                                budget -= ent[3]
                        if budget <= 0:
                            return

                def make_finalize(hl, qw, ctx_ps, ct_tile, hb):
                    def fin():
                        # normalize per 512 chunk: recip of sums row
                        # (psum@base64 -> sbuf@base0), broadcast, multiply
                        for c in range(2):
                            qoff = qw * 1024 + c * 512
                            rc = smpool.tile([1, 512], f32, tag="rc", bufs=1,
                                             name=f"rc{hl}{qw}{c}")
                            nc.vector.reciprocal(rc[0:1, :], ctx_ps[c][64:65, :])
                            rb = smpool.tile([64, 512], f32, tag="rb", bufs=1,
                                             name=f"rb{hl}{qw}{c}")
                            nc.gpsimd.partition_broadcast(rb[:, :], rc[0:1, :])
                            if hb == 0:
                                nc.vector.tensor_mul(
                                    ct_tile[0:64, qoff:qoff + 512],
                                    ctx_ps[c][0:64, :], rb[:, :],
                                )
                            else:
                                stg = smpool.tile([64, 512], bf16, tag="stg",
                                                  bufs=2, name=f"stg{hl}{qw}{c}")
                                nc.vector.tensor_mul(stg[:, :], ctx_ps[c][0:64, :],
                                                     rb[:, :])
                                nc.gpsimd.dma_start(
                                    out=ct_tile[hb:hb + 64, qoff:qoff + 512],
                                    in_=stg[:, :],
                                )
                    return fin

                order = [0, 1, 2, 3, 4, 5, 7, 6]
                blocks = [(hl, qw) for hl in order for qw in range(QW)]
                for bi, (hl, qw) in enumerate(blocks):
                    qh_tile = qhT[hl // 2]
                    kh_tile = khT[hl // 2]
                    hb = (hl % 2) * 64
                    ct_tile = ctxT[hl // 2]
                    if True:
                        ctx_ps = [
                            ctxpspool.tile([65, 512], f32, tag="ctx", bufs=3,
                                           name=f"ctx{hl}{qw}{c}")
                            for c in range(2)
                        ]
                        for st in range(ST):
                            sc_ps = pspool.tile(
                                [128, 1024], f32, tag="sc", bufs=2,
                                name=f"sc{hl}{qw}{st}",
                            )
                            for half in range(2):
                                nc.tensor.matmul(
                                    sc_ps[:, half * 512:(half + 1) * 512],
                                    kh_tile[hb:hb + 64, st * 128:(st + 1) * 128],
                                    qh_tile[hb:hb + 64,
                                            qw * 1024 + half * 512:
                                            qw * 1024 + (half + 1) * 512],
                                    start=True, stop=True,
                                )
                            et = smpool.tile(
                                [128, 1024], bf16, tag="expT", bufs=4,
                                name=f"et{hl}{qw}{st}",
                            )
                            nc.scalar.activation(
                                et[:, :], sc_ps[:, :],
                                mybir.ActivationFunctionType.Exp,
                                scale=float(SCALE),
                            )
                            flush_one_pv()
                            pop_bg(bi)

                            def make_pv(st=st, et=et, ctx_ps=ctx_ps,
                                        vt=vh_tiles[st], hl=hl):
                                def emit():
                                    for half in range(2):
                                        nc.tensor.matmul(
                                            ctx_ps[half][:, :],
                                            vt[:, hl, :],
                                            et[:, half * 512:(half + 1) * 512],
                                            start=(st == 0), stop=(st == ST - 1),
                                        )
                                return emit
                            fin = (make_finalize(hl, qw, ctx_ps, ct_tile, hb)
                                   if st == ST - 1 else None)
                            pending_pv.append((make_pv(), fin))
                while pending_pv:
                    flush_one_pv(depth=1)

            # ---- background streams, strict priority order ----
            # [from_hl, from_qw, gen, skip, pop_weight]; pop_weight 1 =
            # stream may take both pops of a step, 2 = at most one pop.
            def wo_gen(kts, out_ext, scs=(0, 1, 2, 3), alt_pool=False,
                       use_act=False):
                i = 0
                for m in range(8):
                    for sc in scs:
                        yield from wo_chain(m, sc, kts, out_ext,
                                            f"t{kts[0]}", use_act=use_act,
                                            use_ctxps=(alt_pool and i % 2 == 0))
                        i += 1

            # [from_block, gen, skip, pop_weight]; blocks run in order
            # [0,1,2,3,4,5,7,6] x 2 windows, so ctxT[3] rows 64:128 (hl7)
            # finish at blocks 12-13 and the LAST normalize (hl6, block 15)
            # is the direct-write hb=0 path (no gpsimd staging hop on the
            # critical tail).
            bgs = [
                # proj chains; emission deadlines (first PE reader):
                # khT-m0c1 step 8, qhT-m0c1 step 16, m1 step 64,
                # m2 step 128, m3 step 192. 2 pops/step => all done by
                # step 104.
                [0, proj_bg(khT, wt_k, kxt, "bk", 0, (2, 3)), 0, 1],
                [0, proj_bg(qhT, wt_q, qxt, "bq", 0, (2, 3)), 0, 1],
            ]
            for m in range(1, 4):
                bgs.append([0, proj_bg(khT, wt_k, kxt, "bk", m, range(4)), 0, 1])
                bgs.append([0, proj_bg(qhT, wt_q, qxt, "bq", m, range(4)), 0, 1])
            # wo sub-gated per window-half on fin completion positions
            bgs.append([7, wo_gen([0, 1], outA_e, scs=(0, 1)), 2, 2])
            bgs.append([8, wo_gen([0, 1], outA_e, scs=(2, 3)), 0, 2])
            bgs.append([11, wo_gen([2], out2_e, scs=(0, 1)), 2, 2])
            bgs.append([12, wo_gen([2], out2_e, scs=(2, 3)), 0, 2])
            bgs.append([15, wo_gen([3], out3_e, scs=(0, 1), alt_pool=True),
                        5, 1])

            attention(vh_tiles, qhT, khT, bgs=bgs)

            # leftover background (deterministically empty; safety net)
            for ent in bgs:
                if ent[1] is not None:
                    for kind, go in ent[1]:
                        go()
            # rest of the kt=3 partial: per m, two [128,512] chains
            # (sc2 on ACT-drain, sc3 on DVE-drain) into ONE [128,1024]
            # staging tile and ONE output DMA, alternating the SP and
            # gpsimd DGE queues -- the tail end is DMA-issue bound.
            rot = [(pspool, "sc", 2), (ctxpspool, "ctx", 3),
                   (ctxpspool, "ctx", 3), (pspool, "sc", 2),
                   (ctxpspool, "ctx", 3)]
            i = 0
            for m in range(8):
                ot = smpool.tile([128, 1024], bf16, tag="expT", bufs=4,
                                 name=f"tlo{m}")
                for j, sc in enumerate((2, 3)):
                    pool, ptag, pbufs = rot[i % len(rot)]
                    ps = pool.tile([128, 512], f32, tag=ptag, bufs=pbufs,
                                   name=f"tl{m}{sc}")
                    nc.tensor.matmul(
                        ps[:, :],
                        wo_t[3][:, m * 128:(m + 1) * 128],
                        ctxT[3][:, sc * 512:(sc + 1) * 512],
                        start=True, stop=True,
                    )
                    if j == 0:
                        nc.scalar.copy(ot[:, 0:512], ps[:, :])
                    else:
                        nc.vector.tensor_copy(ot[:, 512:1024], ps[:, :])
                    i += 1
                q = nc.sync if m % 2 == 0 else nc.gpsimd
                q.dma_start(
                    out=out3_e[m * 128:(m + 1) * 128, 1024:2048],
                    in_=ot[:, :],
                )

    nc.compile()
    return nc


_NC = None


def _get_program():
    global _NC
    if _NC is None:
        _NC = _build_program()
    return _NC


def make_in_maps(q, k, v, wq, wk, wv, wo, bq, bk):
    bf = ml_dtypes.bfloat16
    in_maps = []
    for b in range(B):
        qT = np.ascontiguousarray(q[b].T.astype(bf))
        kT = np.ascontiguousarray(k[b].T.astype(bf))
        vT = np.ascontiguousarray(v[b].T.astype(bf))
        for g in range(2):
            cols = slice(g * DL, (g + 1) * DL)
            in_maps.append({
                "qT": qT, "kT": kT, "vT": vT,
                "wq": np.ascontiguousarray(wq[:, cols].astype(bf)),
                "wk": np.ascontiguousarray(wk[:, cols].astype(bf)),
                "wv": np.ascontiguousarray(wv[:, cols].astype(bf)),
                "wo": np.ascontiguousarray(wo[cols, :].astype(bf)),
                "bq": np.ascontiguousarray(bq[cols]),
                "bk": np.ascontiguousarray(bk[cols]),
            })
    return in_maps


def assemble_out(results, wo, bv, bo):
    tail = bv @ wo + bo  # exact fold of v/output biases (softmax rows sum to 1)
    out = np.empty((B, S, D), np.float32)
    for b in range(B):
        acc = sum(
            results[2 * b + g][k].astype(np.float32)
            for g in range(2) for k in ("outA", "out2", "out3")
        )
        out[b] = acc.T + tail
    return out


def kernel(q, k, v, wq, bq, wk, bk, wv, bv, wo, bo, **_unused):
    q = np.asarray(q, np.float32)
    k = np.asarray(k, np.float32)
    v = np.asarray(v, np.float32)
    wq = np.asarray(wq, np.float32)
    wk = np.asarray(wk, np.float32)
    wv = np.asarray(wv, np.float32)
    wo = np.asarray(wo, np.float32)
    bq = np.asarray(bq, np.float32)
    bk = np.asarray(bk, np.float32)
    bv = np.asarray(bv, np.float32)
    bo = np.asarray(bo, np.float32)

    nc = _get_program()
    in_maps = make_in_maps(q, k, v, wq, wk, wv, wo, bq, bk)
    res = run_bass_kernel_spmd(nc, in_maps, core_ids=list(range(8))).results
    return assemble_out(res, wo, bv, bo)


if __name__ == "__main__":
    rng = np.random.default_rng(0)
    sd = 1.0 / np.sqrt(D)
    inputs = {
        "q": rng.standard_normal((B, S, D), dtype=np.float32),
        "k": rng.standard_normal((B, S, D), dtype=np.float32),
        "v": rng.standard_normal((B, S, D), dtype=np.float32),
        "wq": rng.standard_normal((D, D), dtype=np.float32) * sd,
        "bq": np.zeros(D, np.float32),
        "wk": rng.standard_normal((D, D), dtype=np.float32) * sd,
        "bk": np.zeros(D, np.float32),
        "wv": rng.standard_normal((D, D), dtype=np.float32) * sd,
        "bv": np.zeros(D, np.float32),
        "wo": rng.standard_normal((D, D), dtype=np.float32) * sd,
        "bo": np.zeros(D, np.float32),
    }
    out = kernel(**inputs)
    print("kernel ran:", out.shape, out.dtype)


# revision 61
# speedup vs baseline: 1.0030x; 1.0030x over previous
"""Multi-head attention on 8 TRN2 NeuronCores (Bass/Tile).

Problem: B=4, S=2048, D=1024, H=16 heads (DH=64).
  out = softmax((q@wq+bq)(k@wk+bk)^T / sqrt(H)) @ (v@wv+bv) @ wo + bo

Sharding: 8 cores = 4 batches x 2 head-groups (8 heads each). Each core
computes its batch's QKV projections restricted to its head group's
columns, attention for those 8 heads, and a partial output projection
(wo rows for its heads); the host sums 3 per-ktile-group partials per
batch. Activations are kept TRANSPOSED ([feature, seq]) on device so
every matmul has its contraction on the partition dim; the host
transposes inputs/outputs (cheap numpy).

v2: PE is the critical engine (~330us busy of matmul work that cannot
shrink at >=16-bit dtypes: QK 109us + PV 109us + QKV projections 82us
+ WO 27us), so the schedule exists to keep PE saturated end-to-end:
  prefix (~40us, paced by bf16 input DMA): vh (all 16 ktiles, all 8
    heads, + ones col) -> khT m-tile 0 chunk 0 -> qhT m-tile 0 chunk 0.
  attention stream, blocks = head x q-window in order
  [0,1,2,3,4,5,7,6] x 2 windows so the LAST block's normalize is the
  direct-write hb=0 path (no gpsimd staging hop on the critical tail):
    scT = khT_h^T @ qhT_h (K=64, PSUM [128,1024]); eT = exp(scT/4)
    (bf16, one ACT instr, ~1.04us = the ACT pace); ctx_ext[65,512] +=
    vh^T @ eT (ones col -> row 64 = softmax sums); normalize via DVE
    reciprocal + gpsimd partition_broadcast + DVE mul -> ctxT bf16.
  Each step pops background PE work into the stream (pop_bg): first
  the remaining projection chains (khT/qhT m0-chunk1 then m-tiles 1-3,
  2 pops/step), then wo partial chains (1/step, bank-cycle limited)
  sub-gated per window-half on normalize completion. Emission-order
  deadlines are hard: PE executes in order, so every chain matmul must
  be EMITTED before the attention matmul that reads its output; with
  2 pops/step all proj chains are emitted by step ~104 vs first-reader
  deadlines {8,16,64,128,192}. Matmul outputs never cross a PSUM bank
  (max 512 f32 cols per accumulation chain).
  Inputs kT/qT stay resident in SBUF (64KB/part) so chains need no
  reloads. All activations and weights are bf16 (PSUM accumulation
  f32); partial outputs outA=wo[kt0]+wo[kt1] (accumulated in PSUM
  before one bf16 round), out2, out3; bv/bo fold exactly through the
  linear tail on the host since softmax rows sum to 1.
  Tail: the 16 post-attention kt3 chains rotate across all 6 freed
  PSUM slots with drains alternating ACT/DVE across two SBUF rings.

The PV matmuls run with a 2-step software-pipeline skew behind the
exps (hardware-validated; depth 4 races on silicon despite simulating
fine, and depth 2 sims 1.7us faster than 3). The ctx ring stays at
3 banks (ring 2 sims 30us slower from PV stalls on normalize).

Measured (8-core SPMD, axon): rel err vs fp32 reference 8.56e-3
(budget 2e-2; bf16 inputs/weights dominate); cost-model timeline
369.4us/core vs 421.6us for the v1 phase-separated schedule
(PE busy ~330/370 = 89%; remaining idle: ~10us DMA-paced prefix,
~10us scattered pop/dead-window bubbles, ~8us tail fin+drain flush).
"""
import ml_dtypes
import numpy as np

import concourse.mybir as mybir
from concourse import bacc
from concourse.tile import TileContext
from concourse.bass_utils import run_bass_kernel_spmd

B, S, D, H = 4, 2048, 1024, 16
DH = D // H          # 64
HG = H // 2          # 8 heads per core
DL = HG * DH         # 512 local qkv width
KT = D // 128        # 8 contraction tiles for projections
ST = S // 128        # 16 key tiles
QW = S // 1024       # 2 q windows of 1024
SC = S // 1024       # 2 s-chunks for input tiles
SCALE = 1.0 / np.sqrt(np.float32(H))  # 0.25

f32 = mybir.dt.float32
bf16 = mybir.dt.bfloat16


def _build_program() -> bacc.Bacc:
    nc = bacc.Bacc()
    qT_e = nc.declare_dram_parameter("qT", [D, S], bf16, isOutput=False)
    kT_e = nc.declare_dram_parameter("kT", [D, S], bf16, isOutput=False)
    vT_e = nc.declare_dram_parameter("vT", [D, S], bf16, isOutput=False)
    wq_e = nc.declare_dram_parameter("wq", [D, DL], bf16, isOutput=False)
    wk_e = nc.declare_dram_parameter("wk", [D, DL], bf16, isOutput=False)
    wv_e = nc.declare_dram_parameter("wv", [D, DL], bf16, isOutput=False)
    wo_e = nc.declare_dram_parameter("wo", [DL, D], bf16, isOutput=False)
    bq_e = nc.declare_dram_parameter("bq", [DL], f32, isOutput=False)
    bk_e = nc.declare_dram_parameter("bk", [DL], f32, isOutput=False)
    # partial outputs: kt0+kt1 merged (chains accumulate both in PSUM
    # before one drain), kt2 and kt3 separate for earlier gating
    outA_e = nc.declare_dram_parameter("outA", [D, S], bf16, isOutput=True)
    out2_e = nc.declare_dram_parameter("out2", [D, S], bf16, isOutput=True)
    out3_e = nc.declare_dram_parameter("out3", [D, S], bf16, isOutput=True)

    with TileContext(nc) as tc:
        with (
            tc.tile_pool(name="wp", bufs=1) as wpool,
            tc.tile_pool(name="kin", bufs=1) as kinpool,
            tc.tile_pool(name="qin", bufs=1) as qinpool,
            tc.tile_pool(name="vin", bufs=1) as vinpool,
            tc.tile_pool(name="proj", bufs=1) as projpool,
            tc.tile_pool(name="attn", bufs=1) as attnpool,
            tc.tile_pool(name="sm", bufs=3) as smpool,
            tc.tile_pool(name="ps", bufs=2, space="PSUM") as pspool,
            tc.tile_pool(name="ctxps", bufs=3, space="PSUM") as ctxpspool,
            tc.tile_pool(name="bgps", bufs=1, space="PSUM") as bgpspool,
        ):
            # ---- DMA queue (SP, FIFO) is ordered by first use: biases,
            # wv, vT, wk, kT-c0, wq, qT-c0, then the bg chunk-1 inputs
            # and wo (not needed until hl4).
            bias_t = {}

            def load_biases():
                # issued after the vT stream: first use is the khT-m0
                # bias-add at ~30us, so keep them off the DMA queue head
                for nm, ext in [("bq", bq_e), ("bk", bk_e)]:
                    bias_t[nm] = wpool.tile([128, DL // 128], f32, tag=nm,
                                            name=f"b_{nm}")
                    nc.sync.dma_start(
                        out=bias_t[nm], in_=ext.rearrange("(j p) -> p j", p=128)
                    )

            def load_input_chunk(pool, nm, ext, c, xt, bufs=SC * KT):
                for t in range(KT):
                    x = pool.tile([128, 1024], bf16, tag="in", bufs=bufs,
                                  name=f"{nm}c{c}t{t}")
                    nc.sync.dma_start(
                        out=x,
                        in_=ext[t * 128:(t + 1) * 128, c * 1024:(c + 1) * 1024],
                    )
                    xt[c][t] = x

            def load_weight_input_interleaved(wnm, wext, pool, xnm, xext, c,
                                              xt, bufs=SC * KT, xq=None):
                # one wide rearranged DMA for the whole weight (1 HWDGE
                # issue instead of 8 in the critical prefix window; same
                # bytes/descriptors), then the input tiles; consumers use
                # identical [128, DL] slices of the wide tile
                xq = xq or nc.sync
                w = wpool.tile([128, KT, DL], bf16, tag=wnm, bufs=1,
                               name=f"w{wnm}")
                nc.sync.dma_start(
                    out=w, in_=wext.rearrange("(t p) d -> p t d", p=128)
                )
                wt = [w[:, t, :] for t in range(KT)]
                for t in range(KT):
                    x = pool.tile([128, 1024], bf16, tag="in", bufs=bufs,
                                  name=f"{xnm}c{c}t{t}")
                    xq.dma_start(
                        out=x,
                        in_=xext[t * 128:(t + 1) * 128, c * 1024:(c + 1) * 1024],
                    )
                    xt[c][t] = x
                return wt

            # vh first: its 27us of PE work starts ~1us in
            vxt = [[None] * KT for _ in range(SC)]
            wt_v = load_weight_input_interleaved("wv", wv_e, vinpool, "vT",
                                                 vT_e, 0, vxt, bufs=14)
            load_input_chunk(vinpool, "vT", vT_e, 1, vxt, bufs=14)
            load_biases()

            vh_tiles = []
            for c in range(SC):
                for sti in range(8):
                    st = c * 8 + sti
                    ps = pspool.tile([128, DL], f32, tag="sc", bufs=2,
                                     name=f"vps{st}")
                    for t in range(KT):
                        nc.tensor.matmul(
                            ps[:, :],
                            vxt[c][t][:, sti * 128:(sti + 1) * 128],
                            wt_v[t][:, :],
                            start=(t == 0), stop=(t == KT - 1),
                        )
                    vt = projpool.tile([128, HG, 65], bf16, tag="vh", bufs=ST,
                                       name=f"vh{st}")
                    nc.vector.tensor_copy(
                        vt[:, :, 0:64], ps.rearrange("p (h d) -> p h d", h=HG)
                    )
                    nc.vector.memset(vt[:, :, 64:65], 1.0)
                    vh_tiles.append(vt)

            khT = [
                projpool.tile([128, S], bf16, tag="khT", bufs=4, name=f"khTm{m}")
                for m in range(4)
            ]
            qhT = [
                projpool.tile([128, S], bf16, tag="qhT", bufs=4, name=f"qhTm{m}")
                for m in range(4)
            ]

            def proj_wide(tiles, wt, xt, bname, m, c):
                # one [128,1024] chunk of a projection dh-tile (prefix path);
                # matmul outputs must stay within one PSUM bank (512 f32),
                # so accumulate the two halves as separate chains
                ps = pspool.tile([128, 1024], f32, tag="sc", bufs=2,
                                 name=f"pp{bname}{m}{c}")
                for half in range(2):
                    for t in range(KT):
                        nc.tensor.matmul(
                            ps[:, half * 512:(half + 1) * 512],
                            wt[t][:, m * 128:(m + 1) * 128],
                            xt[c][t][:, half * 512:(half + 1) * 512],
                            start=(t == 0), stop=(t == KT - 1),
                        )
                nc.vector.tensor_scalar_add(
                    tiles[m][:, c * 1024:(c + 1) * 1024], ps[:, :],
                    bias_t[bname][:, m:m + 1],
                )

            def proj_bg(tiles, wt, xt, bname, m, scs):
                # background chains: [128,512] halves in the 1-bank bg pool
                for sc in scs:
                    c, half = sc // 2, sc % 2
                    state = {}

                    def emit_mm(t, c=c, half=half, m=m, state=state):
                        def go():
                            if t == 0:
                                state["ps"] = bgpspool.tile(
                                    [128, 512], f32, tag="bg", bufs=1,
                                    name=f"bp{bname}{m}{c}{half}")
                            nc.tensor.matmul(
                                state["ps"][:, :],
                                wt[t][:, m * 128:(m + 1) * 128],
                                xt[c][t][:, half * 512:(half + 1) * 512],
                                start=(t == 0), stop=(t == KT - 1),
                            )
                            if t == KT - 1:
                                nc.vector.tensor_scalar_add(
                                    tiles[m][:, sc * 512:(sc + 1) * 512],
                                    state["ps"][:, :],
                                    bias_t[bname][:, m:m + 1],
                                )
                        return go
                    for t in range(KT):
                        yield ("mm", emit_mm(t))

            # kT + khT-m0-c0, qT-c0 + qhT-m0-c0 (attention needs these)
            kxt = [[None] * KT for _ in range(SC)]
            wt_k = load_weight_input_interleaved("wk", wk_e, kinpool, "kT",
                                                 kT_e, 0, kxt)
            proj_wide(khT, wt_k, kxt, "bk", 0, 0)

            qxt = [[None] * KT for _ in range(SC)]
            wt_q = load_weight_input_interleaved("wq", wq_e, qinpool, "qT",
                                                 qT_e, 0, qxt)
            proj_wide(qhT, wt_q, qxt, "bq", 0, 0)

            # chunk-1 inputs + wo: queued behind the prefix DMAs; their
            # consumers pop in during early attention steps
            load_input_chunk(kinpool, "kT", kT_e, 1, kxt)
            load_input_chunk(qinpool, "qT", qT_e, 1, qxt)
            wo_t = [
                wpool.tile([128, D], bf16, tag="wo", bufs=4, name=f"wo{t}")
                for t in range(4)
            ]
            for t in range(4):
                nc.sync.dma_start(out=wo_t[t], in_=wo_e[t * 128:(t + 1) * 128, :])

            # ctxT: heads stacked on partitions, 4 tiles of [128, S]
            ctxT = [
                attnpool.tile([128, S], bf16, tag="ctxT", bufs=4, name=f"ctxT{t}")
                for t in range(4)
            ]

            def wo_chain(m, sc, kts, out_ext, tag, use_act=False,
                         use_ctxps=False):
                # output-projection chain accumulating the given ctxT
                # k-tiles -> one partial drain
                if use_ctxps:
                    ps = ctxpspool.tile([128, 512], f32, tag="ctx", bufs=3,
                                        name=f"bg{tag}{m}{sc}")
                else:
                    ps = bgpspool.tile([128, 512], f32, tag="bg", bufs=1,
                                       name=f"bg{tag}{m}{sc}")
                for i, kt in enumerate(kts):
                    yield ("mm", lambda ps=ps, kt=kt, i=i: nc.tensor.matmul(
                        ps[:, :],
                        wo_t[kt][:, m * 128:(m + 1) * 128],
                        ctxT[kt][:, sc * 512:(sc + 1) * 512],
                        start=(i == 0), stop=(i == len(kts) - 1),
                    ))

                def drain(ps=ps, m=m, sc=sc):
                    ot = smpool.tile([128, 512], bf16, tag="ot", bufs=4,
                                     name=f"ot{tag}{m}{sc}")
                    if use_act:
                        nc.scalar.copy(ot[:, :], ps[:, :])
                    else:
                        nc.vector.tensor_copy(ot[:, :], ps[:, :])
                    nc.sync.dma_start(
                        out=out_ext[m * 128:(m + 1) * 128,
                                    sc * 512:(sc + 1) * 512],
                        in_=ot[:, :],
                    )
                yield ("dma", drain)

            def attention(vh_tiles, qhT, khT, bgs=()):
                pending_pv = []  # (emit_fn, finalize_or_None)

                def flush_one_pv(depth=2):
                    if len(pending_pv) >= depth:
                        emit, fin = pending_pv.pop(0)
                        emit()
                        if fin is not None:
                            fin()

                def pop_bg(bi):
                    budget = 2
                    for ent in bgs:
                        if bi < ent[0] or ent[1] is None:
                            continue
                        if ent[2] > 0:
                            ent[2] -= 1
                            return
                        while budget > 0:
                            try:
                                kind, go = next(ent[1])
                            except StopIteration:
                                ent[1] = None
                                break
                            go()
                            if kind == "mm":
                                budget -= ent[3]
                        if budget <= 0:
                            return

                def make_finalize(hl, qw, ctx_ps, ct_tile, hb):
                    def fin():
                        # normalize per 512 chunk: recip of sums row
                        # (psum@base64 -> sbuf@base0), broadcast, multiply
                        for c in range(2):
                            qoff = qw * 1024 + c * 512
                            rc = smpool.tile([1, 512], f32, tag="rc", bufs=1,
                                             name=f"rc{hl}{qw}{c}")
                            nc.vector.reciprocal(rc[0:1, :], ctx_ps[c][64:65, :])
                            rb = smpool.tile([64, 512], f32, tag="rb", bufs=1,
                                             name=f"rb{hl}{qw}{c}")
                            nc.gpsimd.partition_broadcast(rb[:, :], rc[0:1, :])
                            if hb == 0:
                                nc.vector.tensor_mul(
                                    ct_tile[0:64, qoff:qoff + 512],
                                    ctx_ps[c][0:64, :], rb[:, :],
                                )
                            else:
                                stg = smpool.tile([64, 512], bf16, tag="stg",
                                                  bufs=2, name=f"stg{hl}{qw}{c}")
                                nc.vector.tensor_mul(stg[:, :], ctx_ps[c][0:64, :],
                                                     rb[:, :])
                                nc.gpsimd.dma_start(
                                    out=ct_tile[hb:hb + 64, qoff:qoff + 512],
                                    in_=stg[:, :],
                                )
                    return fin

                order = [0, 1, 2, 3, 4, 5, 7, 6]
                blocks = [(hl, qw) for hl in order for qw in range(QW)]
                for bi, (hl, qw) in enumerate(blocks):
                    qh_tile = qhT[hl // 2]
                    kh_tile = khT[hl // 2]
                    hb = (hl % 2) * 64
                    ct_tile = ctxT[hl // 2]
                    if True:
                        ctx_ps = [
                            ctxpspool.tile([65, 512], f32, tag="ctx", bufs=3,
                                           name=f"ctx{hl}{qw}{c}")
                            for c in range(2)
                        ]
                        for st in range(ST):
                            sc_ps = pspool.tile(
                                [128, 1024], f32, tag="sc", bufs=2,
                                name=f"sc{hl}{qw}{st}",
                            )
                            for half in range(2):
                                nc.tensor.matmul(
                                    sc_ps[:, half * 512:(half + 1) * 512],
                                    kh_tile[hb:hb + 64, st * 128:(st + 1) * 128],
                                    qh_tile[hb:hb + 64,
                                            qw * 1024 + half * 512:
                                            qw * 1024 + (half + 1) * 512],
                                    start=True, stop=True,
                                )
                            et = smpool.tile(
                                [128, 1024], bf16, tag="expT", bufs=4,
                                name=f"et{hl}{qw}{st}",
                            )
                            nc.scalar.activation(
                                et[:, :], sc_ps[:, :],
                                mybir.ActivationFunctionType.Exp,
                                scale=float(SCALE),
                            )
                            flush_one_pv()
                            pop_bg(bi)

                            def make_pv(st=st, et=et, ctx_ps=ctx_ps,
                                        vt=vh_tiles[st], hl=hl):
                                def emit():
                                    for half in range(2):
                                        nc.tensor.matmul(
                                            ctx_ps[half][:, :],
                                            vt[:, hl, :],
                                            et[:, half * 512:(half + 1) * 512],
                                            start=(st == 0), stop=(st == ST - 1),
                                        )
                                return emit
                            fin = (make_finalize(hl, qw, ctx_ps, ct_tile, hb)
                                   if st == ST - 1 else None)
                            pending_pv.append((make_pv(), fin))
                while pending_pv:
                    flush_one_pv(depth=1)

            # ---- background streams, strict priority order ----
            # [from_hl, from_qw, gen, skip, pop_weight]; pop_weight 1 =
            # stream may take both pops of a step, 2 = at most one pop.
            def wo_gen(kts, out_ext, scs=(0, 1, 2, 3), alt_pool=False,
                       use_act=False):
                i = 0
                for m in range(8):
                    for sc in scs:
                        yield from wo_chain(m, sc, kts, out_ext,
                                            f"t{kts[0]}", use_act=use_act,
                                            use_ctxps=(alt_pool and i % 2 == 0))
                        i += 1

            # [from_block, gen, skip, pop_weight]; blocks run in order
            # [0,1,2,3,4,5,7,6] x 2 windows, so ctxT[3] rows 64:128 (hl7)
            # finish at blocks 12-13 and the LAST normalize (hl6, block 15)
            # is the direct-write hb=0 path (no gpsimd staging hop on the
            # critical tail).
            bgs = [
                # proj chains; emission deadlines (first PE reader):
                # khT-m0c1 step 8, qhT-m0c1 step 16, m1 step 64,
                # m2 step 128, m3 step 192. 2 pops/step => all done by
                # step 104.
                [0, proj_bg(khT, wt_k, kxt, "bk", 0, (2, 3)), 0, 1],
                [0, proj_bg(qhT, wt_q, qxt, "bq", 0, (2, 3)), 0, 1],
            ]
            for m in range(1, 4):
                bgs.append([0, proj_bg(khT, wt_k, kxt, "bk", m, range(4)), 0, 1])
                bgs.append([0, proj_bg(qhT, wt_q, qxt, "bq", m, range(4)), 0, 1])
            # wo sub-gated per window-half on fin completion positions
            bgs.append([7, wo_gen([0, 1], outA_e, scs=(0, 1)), 2, 2])
            bgs.append([8, wo_gen([0, 1], outA_e, scs=(2, 3)), 0, 2])
            bgs.append([11, wo_gen([2], out2_e, scs=(0, 1)), 2, 2])
            bgs.append([12, wo_gen([2], out2_e, scs=(2, 3)), 0, 2])
            bgs.append([15, wo_gen([3], out3_e, scs=(0, 1), alt_pool=True),
                        5, 1])

            attention(vh_tiles, qhT, khT, bgs=bgs)

            # leftover background (deterministically empty; safety net)
            for ent in bgs:
                if ent[1] is not None:
                    for kind, go in ent[1]:
                        go()
            # rest of the kt=3 partial: per m, two [128,512] chains
            # (sc2 on ACT-drain, sc3 on DVE-drain) into ONE [128,1024]
            # staging tile and ONE output DMA, alternating the SP and
            # gpsimd DGE queues -- the tail end is DMA-issue bound.
            rot = [(pspool, "sc", 2), (ctxpspool, "ctx", 3),
                   (ctxpspool, "ctx", 3), (pspool, "sc", 2),
                   (ctxpspool, "ctx", 3)]
            i = 0
            for m in range(8):
                ot = smpool.tile([128, 1024], bf16, tag="expT", bufs=4,
                                 name=f"tlo{m}")
                for j, sc in enumerate((2, 3)):
                    pool, ptag, pbufs = rot[i % len(rot)]
                    ps = pool.tile([128, 512], f32, tag=ptag, bufs=pbufs,
                                   name=f"tl{m}{sc}")
                    nc.tensor.matmul(
                        ps[:, :],
                        wo_t[3][:, m * 128:(m + 1) * 128],
                        ctxT[3][:, sc * 512:(sc + 1) * 512],
                        start=True, stop=True,
                    )
                    if j == 0:
                        nc.scalar.copy(ot[:, 0:512], ps[:, :])
                    else:
                        nc.vector.tensor_copy(ot[:, 512:1024], ps[:, :])
                    i += 1
                q = nc.sync if m % 2 == 0 else nc.gpsimd
                q.dma_start(
                    out=out3_e[m * 128:(m + 1) * 128, 1024:2048],
                    in_=ot[:, :],
                )

    nc.compile()
    return nc


_NC = None


def _get_program():
    global _NC
    if _NC is None:
        _NC = _build_program()
    return _NC


def make_in_maps(q, k, v, wq, wk, wv, wo, bq, bk):
    bf = ml_dtypes.bfloat16
    in_maps = []
    for b in range(B):
        qT = np.ascontiguousarray(q[b].T.astype(bf))
        kT = np.ascontiguousarray(k[b].T.astype(bf))
        vT = np.ascontiguousarray(v[b].T.astype(bf))
        for g in range(2):
            cols = slice(g * DL, (g + 1) * DL)
            in_maps.append({
                "qT": qT, "kT": kT, "vT": vT,
                "wq": np.ascontiguousarray(wq[:, cols].astype(bf)),
                "wk": np.ascontiguousarray(wk[:, cols].astype(bf)),
                "wv": np.ascontiguousarray(wv[:, cols].astype(bf)),
                "wo": np.ascontiguousarray(wo[cols, :].astype(bf)),
                "bq": np.ascontiguousarray(bq[cols]),
                "bk": np.ascontiguousarray(bk[cols]),
            })
    return in_maps


def assemble_out(results, wo, bv, bo):
    tail = bv @ wo + bo  # exact fold of v/output biases (softmax rows sum to 1)
    out = np.empty((B, S, D), np.float32)
    for b in range(B):
        acc = sum(
            results[2 * b + g][k].astype(np.float32)
            for g in range(2) for k in ("outA", "out2", "out3")
        )
        out[b] = acc.T + tail
    return out


def kernel(q, k, v, wq, bq, wk, bk, wv, bv, wo, bo, **_unused):
    q = np.asarray(q, np.float32)
    k = np.asarray(k, np.float32)
    v = np.asarray(v, np.float32)
    wq = np.asarray(wq, np.float32)
    wk = np.asarray(wk, np.float32)
    wv = np.asarray(wv, np.float32)
    wo = np.asarray(wo, np.float32)
    bq = np.asarray(bq, np.float32)
    bk = np.asarray(bk, np.float32)
    bv = np.asarray(bv, np.float32)
    bo = np.asarray(bo, np.float32)

    nc = _get_program()
    in_maps = make_in_maps(q, k, v, wq, wk, wv, wo, bq, bk)
    res = run_bass_kernel_spmd(nc, in_maps, core_ids=list(range(8))).results
    return assemble_out(res, wo, bv, bo)


if __name__ == "__main__":
    rng = np.random.default_rng(0)
    sd = 1.0 / np.sqrt(D)
    inputs = {
        "q": rng.standard_normal((B, S, D), dtype=np.float32),
        "k": rng.standard_normal((B, S, D), dtype=np.float32),
        "v": rng.standard_normal((B, S, D), dtype=np.float32),
        "wq": rng.standard_normal((D, D), dtype=np.float32) * sd,
        "bq": np.zeros(D, np.float32),
        "wk": rng.standard_normal((D, D), dtype=np.float32) * sd,
        "bk": np.zeros(D, np.float32),
        "wv": rng.standard_normal((D, D), dtype=np.float32) * sd,
        "bv": np.zeros(D, np.float32),
        "wo": rng.standard_normal((D, D), dtype=np.float32) * sd,
        "bo": np.zeros(D, np.float32),
    }
    out = kernel(**inputs)
    print("kernel ran:", out.shape, out.dtype)
